# revision 13
# baseline (speedup 1.0000x reference)
"""Multi-head causal attention (B=8, S=1024, D=768, H=12) on 8 trn2 NeuronCores.

Strategy: data-parallel over batch (one batch element per core, no collectives).

Per-core dataflow (all matmuls bf16 into fp32 PSUM):
  - host passes x^T and all weights pre-cast to bf16; Q^T/K^T via transposed
    projection (W stationary, x^T moving), V via natural projection (x^T
    stationary, W_v moving) -> no on-device transposes.
  - attention as S^T[k,q] = K @ Q^T per head; the two heads of a 128-row
    group go to the two 512-column halves of one PSUM tile (tile_position
    row packing).
  - causal handling: for diagonal-crossing key blocks the fully-masked low
    query columns are skipped in BOTH the QK^T and A@V matmuls (N-width
    trim), exp covers only the valid span, and just the [128,128] diagonal
    triangle gets a bf16 mask multiply (in place).
  - kc-granular software pipeline: QK^T(kc) ... A@V(kc-2) keeps the PE fed
    while ScalarE exponentiates; Q/K-proj, V-proj and output-proj work units
    are woven into the remaining gaps (matmuls first, PSUM->SBUF finish
    copies deferred to the next slot) so the PE never idles and its p-state
    stays at max clock.
  - softmax: exp straight out of PSUM (1/8 scale folded into W_q host-side;
    scores are small, no max-subtraction); denominator free via a ones
    column appended to V (row 64 of the A@V PSUM); fast reciprocal from
    PSUM on DVE; partition broadcast on GpSimd (no DRAM round-trip);
    division as an all-SBUF bf16 tensor-tensor on GpSimd into out^T.
"""
import sys

if "/opt/trn_rl_repo" not in sys.path:
    sys.path.insert(0, "/opt/trn_rl_repo")

import numpy as np

B, S, D, H = 8, 1024, 768, 12
DH = 64
NC_ = 8
NT = D // 128    # 6
ST = S // 128    # 8
QC = S // 512    # 2
VPW = H * (DH + 1)  # 780

_compiled = None
DEBUG_DUMP = False


def _build_masks():
    import ml_dtypes

    i = np.arange(128)[:, None, None]
    t = np.arange(4)[None, :, None]
    j = np.arange(512)[None, None, :]
    m = ((128 * t + i) <= j).astype(np.float32)
    return m.astype(ml_dtypes.bfloat16)


def _build_nc():
    import concourse.bass as bass
    import concourse.mybir as mybir
    import concourse.tile as tile
    from concourse import bacc

    F32 = mybir.dt.float32
    BF16 = mybir.dt.bfloat16
    AF = mybir.ActivationFunctionType
    MULT = mybir.AluOpType.mult

    nc = bacc.Bacc("TRN2", target_bir_lowering=False, debug=False)

    xT_d = nc.dram_tensor("xT", [D, S], BF16, kind="ExternalInput")
    wq_d = nc.dram_tensor("wq", [D, D], BF16, kind="ExternalInput")
    wk_d = nc.dram_tensor("wk", [D, D], BF16, kind="ExternalInput")
    wv_d = nc.dram_tensor("wv", [D, D], BF16, kind="ExternalInput")
    wp_d = nc.dram_tensor("wp", [D, D], BF16, kind="ExternalInput")
    mask_d = nc.dram_tensor("masks", [128, 4, 512], BF16, kind="ExternalInput")
    y_d = nc.dram_tensor("y", [S, D], F32, kind="ExternalOutput")
    if DEBUG_DUMP:
        dbg_q = nc.dram_tensor("dbg_q", [128, NT, S], BF16, kind="ExternalOutput")
        dbg_k = nc.dram_tensor("dbg_k", [128, NT, S], BF16, kind="ExternalOutput")
        dbg_v = nc.dram_tensor("dbg_v", [128, ST, VPW], BF16, kind="ExternalOutput")
        dbg_o = nc.dram_tensor("dbg_o", [128, NT, S], BF16, kind="ExternalOutput")

    with tile.TileContext(nc) as tc:
        with (
            tc.tile_pool(name="static", bufs=1) as static,
            tc.tile_pool(name="pt", bufs=6) as ptp,
            tc.tile_pool(name="mh", bufs=10) as mhp,
            tc.tile_pool(name="pu", bufs=4) as pup,
            tc.tile_pool(name="rc", bufs=4) as rcp,
            tc.tile_pool(name="rb", bufs=4) as rbp,
            tc.tile_pool(name="ysb", bufs=2) as ysbp,
            tc.tile_pool(name="psb", bufs=2, space="PSUM") as psb,
            tc.tile_pool(name="po", bufs=2, space="PSUM") as pop,
            tc.tile_pool(name="psh", bufs=2, space="PSUM") as psh,
        ):
            # ---- persistent SBUF ----
            xT = static.tile([128, NT, S], BF16)
            qT = static.tile([128, NT, S], BF16)
            kT = static.tile([128, NT, S], BF16)
            vp = static.tile([128, ST, VPW], BF16)
            outT = static.tile([128, NT, S], BF16)
            msk = static.tile([128, 4, 512], BF16)
            wv_sb = static.tile([128, NT, D], BF16)
            wk_sb = static.tile([128, NT, D], BF16)
            wq_sb = static.tile([128, NT, D], BF16)
            wp_sb = static.tile([128, NT, D], BF16)

            def chunked_src(t_d, inner):
                full = t_d[:, :]
                return bass.AP(tensor=full.tensor, offset=full.offset,
                               ap=[[inner, 128], [128 * inner, NT], [1, inner]])

            nc.sync.dma_start(xT[:], chunked_src(xT_d, S))
            nc.sync.dma_start(wv_sb[:], chunked_src(wv_d, D))
            nc.sync.dma_start(wk_sb[:], chunked_src(wk_d, D))
            nc.sync.dma_start(wq_sb[:], chunked_src(wq_d, D))
            nc.sync.dma_start(msk[:], mask_d[:])
            nc.sync.dma_start(wp_sb[:], chunked_src(wp_d, D))

            # ones columns of vp ride along A@V as the softmax denominator
            nc.gpsimd.memset(vp[:], 1.0)

            # ---- work units: emit matmuls now, return a finish closure ----
            def VP(st, half, fin_eng):
                ps = psh.tile([128, 512], F32, tag="psh", name=f"vps{st}h{half}")
                c0 = 512 * half
                w = 512 if half == 0 else 256
                for dc in range(NT):
                    nc.tensor.matmul(
                        ps[:, 0:w], xT[:, dc, 128 * st:128 * (st + 1)],
                        wv_sb[:, dc, c0:c0 + w],
                        start=(dc == 0), stop=(dc == NT - 1))
                dst = vp[:, st, :].rearrange("p (h e) -> p h e", e=DH + 1)

                def fin():
                    src = ps[:, 0:w].rearrange("p (h d) -> p h d", d=DH)
                    if half == 0:
                        o = dst[:, 0:8, 0:DH]
                    else:
                        o = dst[:, 8:12, 0:DH]
                    if fin_eng == "s":
                        nc.scalar.activation(o, src, AF.Copy)
                    else:
                        nc.vector.tensor_copy(out=o, in_=src)
                return fin

            def PJ(w_sb, dstT, nt, sc, fin_eng):
                ps = psh.tile([128, 512], F32, tag="psh", name=f"pj{nt}_{sc}")
                for dc in range(NT):
                    nc.tensor.matmul(
                        ps[:], w_sb[:, dc, 128 * nt:128 * (nt + 1)],
                        xT[:, dc, 512 * sc:512 * (sc + 1)],
                        start=(dc == 0), stop=(dc == NT - 1))

                def fin():
                    o = dstT[:, nt, 512 * sc:512 * (sc + 1)]
                    if fin_eng == "s":
                        nc.scalar.activation(o, ps[:], AF.Copy)
                    else:
                        nc.vector.tensor_copy(out=o, in_=ps[:])
                return fin

            ysb_tiles = {}

            def SE(st, half, fin_eng):
                if half == 0:
                    ysb_tiles[st] = ysbp.tile([128, D], F32, tag="ysb",
                                              name=f"ysb{st}")
                ysb = ysb_tiles[st]
                ps = psh.tile([128, 512], F32, tag="psh", name=f"se{st}h{half}")
                c0 = 512 * half
                w = 512 if half == 0 else 256
                for dc in range(NT):
                    nc.tensor.matmul(
                        ps[:, 0:w], outT[:, dc, 128 * st:128 * (st + 1)],
                        wp_sb[:, dc, c0:c0 + w],
                        start=(dc == 0), stop=(dc == NT - 1))

                def fin():
                    if fin_eng == "s":
                        nc.scalar.activation(ysb[:, c0:c0 + w], ps[:, 0:w],
                                             AF.Copy)
                    else:
                        nc.vector.tensor_copy(out=ysb[:, c0:c0 + w],
                                              in_=ps[:, 0:w])
                    if half == 1:
                        nc.sync.dma_start(y_d[128 * st:128 * (st + 1), :],
                                          ysb[:])
                return fin

            # ---- filler scheduler ----
            fillers = []
            pend_fin = []

            def flush_fins():
                while pend_fin:
                    pend_fin.pop(0)()

            def pop_fill(n=1):
                for _ in range(n):
                    flush_fins()
                    if fillers:
                        fin = fillers.pop(0)[1]()
                        pend_fin.append(fin)

            def force(*keys):
                # deadline path: unit's finish copy must land before the
                # consumer instructions that follow, so emit it immediately
                for key in keys:
                    for i, (k, fn) in enumerate(fillers):
                        if k == key:
                            flush_fins()
                            fillers.pop(i)
                            fn()()
                            break

            def mk_vp(st, half, e):
                return (f"vp{st}h{half}", lambda: VP(st, half, e))

            def mk_pj(kind, nt, sc, e):
                if kind == "q":
                    return (f"q{nt}s{sc}", lambda: PJ(wq_sb, qT, nt, sc, e))
                return (f"k{nt}s{sc}", lambda: PJ(wk_sb, kT, nt, sc, e))

            def mk_se(st, half, e):
                return (f"se{st}h{half}", lambda: SE(st, half, e))

            # ---- attention ----
            pend_div = []   # (outT slice args, pu, rb) deferred to next group

            def flush_divs():
                while pend_div:
                    hp_, qc_, hh_, pu_, rb_ = pend_div.pop(0)
                    nc.gpsimd.tensor_tensor(
                        outT[64 * hh_:64 * (hh_ + 1), hp_,
                             512 * qc_:512 * (qc_ + 1)],
                        pu_[0:64, :], rb_[:], MULT)

            pend_pu = []    # po -> pu copies deferred to next group start

            def flush_pus():
                while pend_pu:
                    pend_pu.pop(0)()

            def emit_att(qc, hp, slots):
                K = 4 * (qc + 1)
                pts = [None] * K
                offs = [None] * K
                po_t = {}

                flush_pus()

                def qkt(kc):
                    t = kc - 4 * qc
                    off = 128 * t if 0 <= t <= 3 else 0
                    offs[kc] = off
                    w = 512 - off
                    ps = psb.tile([128, 1024], F32, tag="big",
                                  name=f"s_{qc}_{hp}_{kc}")
                    for hh in range(2):
                        rows = slice(64 * hh, 64 * (hh + 1))
                        nc.tensor.matmul(
                            ps[:, 512 * hh + off:512 * (hh + 1)],
                            kT[rows, hp, 128 * kc:128 * (kc + 1)],
                            qT[rows, hp, 512 * qc + off:512 * (qc + 1)],
                            start=True, stop=True,
                            tile_position=(64 * hh, 0))
                    if 0 <= t <= 3:
                        mvs = []
                        for hh in range(2):
                            mh = mhp.tile([128, 512], BF16, tag="mh")
                            nc.scalar.activation(
                                mh[:, off:512],
                                ps[:, 512 * hh + off:512 * (hh + 1)], AF.Exp)
                            tri = slice(off, off + 128)
                            nc.vector.tensor_tensor(
                                mh[:, tri], mh[:, tri], msk[:, t, tri], MULT)
                            mvs.append((mh, 0))
                        pts[kc] = mvs
                    else:
                        pt = ptp.tile([128, 1024], BF16, tag="pt")
                        nc.scalar.activation(pt[:], ps[:], AF.Exp)
                        pts[kc] = [(pt, 0), (pt, 512)]

                def av(kc):
                    off = offs[kc]
                    for hh in range(2):
                        if hh not in po_t:
                            po_t[hh] = pop.tile([65, 512], F32, tag="po",
                                                name=f"po_{qc}_{hp}_{hh}")
                        h = 2 * hp + hh
                        src, c0 = pts[kc][hh]
                        nc.tensor.matmul(
                            po_t[hh][:, off:512],
                            vp[:, kc, 65 * h:65 * (h + 1)],
                            src[:, c0 + off:c0 + 512],
                            start=(kc == 0), stop=(kc == K - 1))

                LAG = 2
                for kc in range(K):
                    if kc >= LAG:
                        av(kc - LAG)
                    qkt(kc)
                    if kc == 1:
                        flush_divs()
                    if kc in slots:
                        pop_fill(slots[kc])
                flush_fins()  # vp writes must precede the trailing A@V reads
                av(K - 2)
                av(K - 1)

                for hh in range(2):
                    # pu copy frees po fast.  DVE lanes are partition-locked,
                    # so the den row (partition 64) is moved to partition 0
                    # by GpSimd before the reciprocal.
                    pu = pup.tile([65, 512], F32, tag="pu")
                    po_hh = po_t[hh]
                    nc.vector.tensor_copy(out=pu[:], in_=po_hh[:])
                    rb = rbp.tile([64, 512], F32, tag="rb")

                    def den_fin(pu_=pu, rb_=rb):
                        dn = rcp.tile([1, 512], F32, tag="rc", name="dn")
                        nc.gpsimd.tensor_copy(out=dn[:], in_=pu_[64:65, :])
                        rc = rcp.tile([1, 512], F32, tag="rc", name="rc")
                        nc.vector.reciprocal_approx_fast(
                            out=rc[:], in_=dn[:])
                        nc.gpsimd.partition_broadcast(rb_[:], rc[:],
                                                      channels=64)
                    pend_pu.append(den_fin)
                    pend_div.append((hp, qc, hh, pu, rb))

            # ---- phase A: warm-up ----
            for st, half in ((0, 0), (0, 1), (1, 0), (1, 1)):
                VP(st, half, "v")()
            PJ(wk_sb, kT, 0, 0, "v")()
            PJ(wk_sb, kT, 0, 1, "v")()
            PJ(wq_sb, qT, 0, 0, "v")()

            fillers += [mk_vp(2, 0, "s"), mk_vp(2, 1, "s"),
                        mk_vp(3, 0, "s"), mk_vp(3, 1, "s")]
            for nt in range(1, NT):
                fillers += [mk_pj("q", nt, 0, "s"), mk_pj("k", nt, 0, "s"),
                            mk_pj("k", nt, 1, "s")]
            fillers += [mk_vp(st, h, "s") for st in (4, 5, 6, 7)
                        for h in (0, 1)]
            fillers += [mk_pj("q", nt, 1, "s") for nt in range(NT)]

            # ---- qc0 pass ----
            for hp in range(NT):
                if hp > 0:
                    force(f"q{hp}s0", f"k{hp}s0", f"k{hp}s1")
                emit_att(0, hp, slots={0: 1, 1: 2, 2: 1, 3: 1})

            # SE units become available once qc0 outT is complete
            fillers += [mk_se(st, h, "v") for st in (0, 1, 2, 3)
                        for h in (0, 1)]

            # ---- qc1 pass ----
            for hp in range(NT):
                force(f"q{hp}s1")
                emit_att(1, hp, slots={1: 1, 3: 1, 5: 1})

            # ---- tail ----
            while fillers:
                pop_fill(1)
            flush_fins()
            flush_pus()
            flush_divs()
            eng = ["v", "s"]
            for i, (st, half) in enumerate(
                    [(st, h) for st in (4, 5, 6, 7) for h in (0, 1)]):
                fin = SE(st, half, eng[i % 2])
                flush_fins()
                pend_fin.append(fin)
            flush_fins()
            if DEBUG_DUMP:
                nc.sync.dma_start(dbg_q[:], qT[:])
                nc.sync.dma_start(dbg_k[:], kT[:])
                nc.sync.dma_start(dbg_v[:], vp[:])
                nc.sync.dma_start(dbg_o[:], outT[:])

    nc.compile()
    return nc


def _get_compiled():
    global _compiled
    if _compiled is None:
        _compiled = _build_nc()
    return _compiled


def _prep_inputs(x, W_attn, W_proj):
    import ml_dtypes

    bf16 = ml_dtypes.bfloat16
    x = np.asarray(x, dtype=np.float32)
    W_attn = np.asarray(W_attn, dtype=np.float32)
    W_proj = np.asarray(W_proj, dtype=np.float32)

    xT = np.ascontiguousarray(np.transpose(x, (0, 2, 1))).astype(bf16)
    wq = (np.ascontiguousarray(W_attn[:, 0:D]) * np.float32(0.125)).astype(bf16)
    wk = np.ascontiguousarray(W_attn[:, D:2 * D]).astype(bf16)
    wv = np.ascontiguousarray(W_attn[:, 2 * D:3 * D]).astype(bf16)
    wp = W_proj.astype(bf16)
    masks = _build_masks()
    return [
        {"xT": xT[b], "wq": wq, "wk": wk, "wv": wv, "wp": wp, "masks": masks}
        for b in range(B)
    ]


def kernel(x, W_attn, W_proj):
    from concourse.bass_utils import run_bass_kernel_spmd

    nc = _get_compiled()
    in_maps = _prep_inputs(x, W_attn, W_proj)
    res = run_bass_kernel_spmd(nc, in_maps, list(range(NC_)))
    y = np.stack([res.results[b]["y"] for b in range(B)], axis=0)
    return y.astype(np.float32)


# revision 20
# speedup vs baseline: 2.9107x; 2.9107x over previous
"""Multi-head causal attention (B=8, S=1024, D=768, H=12) on 8 trn2 NeuronCores.

Strategy: data-parallel over batch (one batch element per core, no collectives).

Per-core dataflow (all matmuls bf16 into fp32 PSUM):
  - host passes x^T and all weights pre-cast to bf16; Q^T/K^T via transposed
    projection (W stationary, x^T moving), V via natural projection (x^T
    stationary, W_v moving) -> no on-device transposes.
  - attention as S^T[k,q] = K @ Q^T per head; the two heads of a 128-row
    group go to the two 512-column halves of one PSUM tile (tile_position
    row packing).
  - causal handling: for diagonal-crossing key blocks the fully-masked low
    query columns are skipped in BOTH the QK^T and A@V matmuls (N-width
    trim), exp covers only the valid span, and just the [128,128] diagonal
    triangle gets a bf16 mask multiply (in place).
  - kc-granular software pipeline: QK^T(kc) ... A@V(kc-2) keeps the PE fed
    while ScalarE exponentiates; Q/K-proj, V-proj and output-proj work units
    are woven into the remaining gaps (matmuls first, PSUM->SBUF finish
    copies deferred to the next slot) so the PE never idles and its p-state
    stays at max clock.
  - softmax: exp straight out of PSUM (1/8 scale folded into W_q host-side;
    scores are small, no max-subtraction); denominator free via a ones
    column appended to V (row 64 of the A@V PSUM); fast reciprocal from
    PSUM on DVE; partition broadcast on GpSimd (no DRAM round-trip);
    division as an all-SBUF bf16 tensor-tensor on GpSimd into out^T.
"""
import sys

if "/opt/trn_rl_repo" not in sys.path:
    sys.path.insert(0, "/opt/trn_rl_repo")

import numpy as np

B, S, D, H = 8, 1024, 768, 12
DH = 64
NC_ = 8
NT = D // 128    # 6
ST = S // 128    # 8
QC = S // 512    # 2
VPW = H * (DH + 1)  # 780

_compiled = None
DEBUG_DUMP = False


def _build_masks():
    import ml_dtypes

    i = np.arange(128)[:, None, None]
    t = np.arange(4)[None, :, None]
    j = np.arange(512)[None, None, :]
    m = ((128 * t + i) <= j).astype(np.float32)
    return m.astype(ml_dtypes.bfloat16)


def _build_nc():
    import concourse.bass as bass
    import concourse.mybir as mybir
    import concourse.tile as tile
    from concourse import bacc

    F32 = mybir.dt.float32
    BF16 = mybir.dt.bfloat16
    AF = mybir.ActivationFunctionType
    MULT = mybir.AluOpType.mult

    nc = bacc.Bacc("TRN2", target_bir_lowering=False, debug=False)

    xT_d = nc.dram_tensor("xT", [D, S], BF16, kind="ExternalInput")
    wq_d = nc.dram_tensor("wq", [D, D], BF16, kind="ExternalInput")
    wk_d = nc.dram_tensor("wk", [D, D], BF16, kind="ExternalInput")
    wv_d = nc.dram_tensor("wv", [D, D], BF16, kind="ExternalInput")
    wp_d = nc.dram_tensor("wp", [D, D], BF16, kind="ExternalInput")
    mask_d = nc.dram_tensor("masks", [128, 4, 512], BF16, kind="ExternalInput")
    y_d = nc.dram_tensor("y", [S, D], F32, kind="ExternalOutput")
    scr_d = nc.dram_tensor("den_scratch", [NT, QC, 2, 512], F32)
    if DEBUG_DUMP:
        dbg_q = nc.dram_tensor("dbg_q", [128, NT, S], BF16, kind="ExternalOutput")
        dbg_k = nc.dram_tensor("dbg_k", [128, NT, S], BF16, kind="ExternalOutput")
        dbg_v = nc.dram_tensor("dbg_v", [128, ST, VPW], BF16, kind="ExternalOutput")
        dbg_o = nc.dram_tensor("dbg_o", [128, NT, S], BF16, kind="ExternalOutput")

    with tile.TileContext(nc) as tc:
        with (
            tc.tile_pool(name="static", bufs=1) as static,
            tc.tile_pool(name="pt", bufs=6) as ptp,
            tc.tile_pool(name="mh", bufs=10) as mhp,
            tc.tile_pool(name="pu", bufs=4) as pup,
            tc.tile_pool(name="rb", bufs=4) as rbp,
            tc.tile_pool(name="rr", bufs=4) as rrp,
            tc.tile_pool(name="dv", bufs=4) as dvp,
            tc.tile_pool(name="ysb", bufs=2) as ysbp,
            tc.tile_pool(name="psb", bufs=2, space="PSUM") as psb,
            tc.tile_pool(name="po", bufs=2, space="PSUM") as pop,
            tc.tile_pool(name="psh", bufs=2, space="PSUM") as psh,
        ):
            # ---- persistent SBUF ----
            xT = static.tile([128, NT, S], BF16)
            qT = static.tile([128, NT, S], BF16)
            kT = static.tile([128, NT, S], BF16)
            vp = static.tile([128, ST, VPW], BF16)
            outT = static.tile([128, NT, S], BF16)
            msk = static.tile([128, 4, 512], BF16)
            wv_sb = static.tile([128, NT, D], BF16)
            wk_sb = static.tile([128, NT, D], BF16)
            wq_sb = static.tile([128, NT, D], BF16)
            wp_sb = static.tile([128, NT, D], BF16)

            def chunked_src(t_d, inner):
                full = t_d[:, :]
                return bass.AP(tensor=full.tensor, offset=full.offset,
                               ap=[[inner, 128], [128 * inner, NT], [1, inner]])

            nc.sync.dma_start(xT[:], chunked_src(xT_d, S))
            nc.sync.dma_start(wv_sb[:], chunked_src(wv_d, D))
            nc.sync.dma_start(wk_sb[:], chunked_src(wk_d, D))
            nc.sync.dma_start(wq_sb[:], chunked_src(wq_d, D))
            nc.sync.dma_start(msk[:], mask_d[:])
            nc.sync.dma_start(wp_sb[:], chunked_src(wp_d, D))

            # ones columns of vp ride along A@V as the softmax denominator
            nc.gpsimd.memset(vp[:], 1.0)

            # ---- work units: emit matmuls now, return a finish closure ----
            def VP(st, half, fin_eng):
                ps = psh.tile([128, 512], F32, tag="psh", name=f"vps{st}h{half}")
                c0 = 512 * half
                w = 512 if half == 0 else 256
                for dc in range(NT):
                    nc.tensor.matmul(
                        ps[:, 0:w], xT[:, dc, 128 * st:128 * (st + 1)],
                        wv_sb[:, dc, c0:c0 + w],
                        start=(dc == 0), stop=(dc == NT - 1))
                dst = vp[:, st, :].rearrange("p (h e) -> p h e", e=DH + 1)

                def fin():
                    src = ps[:, 0:w].rearrange("p (h d) -> p h d", d=DH)
                    if half == 0:
                        o = dst[:, 0:8, 0:DH]
                    else:
                        o = dst[:, 8:12, 0:DH]
                    if fin_eng == "s":
                        nc.scalar.activation(o, src, AF.Copy)
                    else:
                        nc.vector.tensor_copy(out=o, in_=src)
                return fin

            def PJ(w_sb, dstT, nt, sc, fin_eng):
                ps = psh.tile([128, 512], F32, tag="psh", name=f"pj{nt}_{sc}")
                for dc in range(NT):
                    nc.tensor.matmul(
                        ps[:], w_sb[:, dc, 128 * nt:128 * (nt + 1)],
                        xT[:, dc, 512 * sc:512 * (sc + 1)],
                        start=(dc == 0), stop=(dc == NT - 1))

                def fin():
                    o = dstT[:, nt, 512 * sc:512 * (sc + 1)]
                    if fin_eng == "s":
                        nc.scalar.activation(o, ps[:], AF.Copy)
                    else:
                        nc.vector.tensor_copy(out=o, in_=ps[:])
                return fin

            ysb_tiles = {}

            def SE(st, half, fin_eng):
                if half == 0:
                    ysb_tiles[st] = ysbp.tile([128, D], F32, tag="ysb",
                                              name=f"ysb{st}")
                ysb = ysb_tiles[st]
                ps = psh.tile([128, 512], F32, tag="psh", name=f"se{st}h{half}")
                c0 = 512 * half
                w = 512 if half == 0 else 256
                for dc in range(NT):
                    nc.tensor.matmul(
                        ps[:, 0:w], outT[:, dc, 128 * st:128 * (st + 1)],
                        wp_sb[:, dc, c0:c0 + w],
                        start=(dc == 0), stop=(dc == NT - 1))

                def fin():
                    if fin_eng == "s":
                        nc.scalar.activation(ysb[:, c0:c0 + w], ps[:, 0:w],
                                             AF.Copy)
                    else:
                        nc.vector.tensor_copy(out=ysb[:, c0:c0 + w],
                                              in_=ps[:, 0:w])
                    if half == 1:
                        nc.sync.dma_start(y_d[128 * st:128 * (st + 1), :],
                                          ysb[:])
                return fin

            # ---- filler scheduler ----
            fillers = []
            pend_fin = []

            def flush_fins():
                while pend_fin:
                    pend_fin.pop(0)()

            def pop_fill(n=1):
                for _ in range(n):
                    flush_fins()
                    if fillers:
                        fin = fillers.pop(0)[1]()
                        pend_fin.append(fin)

            def force(*keys):
                # deadline path: unit's finish copy must land before the
                # consumer instructions that follow, so emit it immediately
                for key in keys:
                    for i, (k, fn) in enumerate(fillers):
                        if k == key:
                            flush_fins()
                            fillers.pop(i)
                            fn()()
                            break

            def mk_vp(st, half, e):
                return (f"vp{st}h{half}", lambda: VP(st, half, e))

            def mk_pj(kind, nt, sc, e):
                if kind == "q":
                    return (f"q{nt}s{sc}", lambda: PJ(wq_sb, qT, nt, sc, e))
                return (f"k{nt}s{sc}", lambda: PJ(wk_sb, kT, nt, sc, e))

            def mk_se(st, half, e):
                return (f"se{st}h{half}", lambda: SE(st, half, e))

            # ---- attention ----
            # denominator path: pu copy (frees po) + DMA round-trip that
            # shifts the den row to partition 0 AND broadcasts it to 64
            # partitions; reciprocal + division on DVE are deferred into
            # the next group (they are slack-tolerant).
            pend_den = []

            def flush_den():
                while pend_den:
                    hp_, qc_, hh_, pu_, rb_ = pend_den.pop(0)
                    rr = rrp.tile([64, 512], F32, tag="rr", name="rr")
                    nc.vector.reciprocal_approx_fast(out=rr[:], in_=rb_[:])
                    dst = outT[64 * hh_:64 * (hh_ + 1), hp_,
                               512 * qc_:512 * (qc_ + 1)]
                    if hh_ == 0:
                        nc.vector.tensor_tensor(dst, pu_[0:64, :], rr[:],
                                                MULT)
                    else:
                        # DVE lanes are partition-locked: write rows 64-127
                        # via a small tile + partition-shifting local DMA
                        dv = dvp.tile([64, 512], BF16, tag="dv", name="dv")
                        nc.vector.tensor_tensor(dv[:], pu_[0:64, :], rr[:],
                                                MULT)
                        nc.sync.dma_start(dst, dv[:])

            def emit_att(qc, hp, slots):
                K = 4 * (qc + 1)
                pts = [None] * K
                offs = [None] * K
                po_t = {}

                def qkt(kc):
                    t = kc - 4 * qc
                    off = 128 * t if 0 <= t <= 3 else 0
                    offs[kc] = off
                    w = 512 - off
                    ps = psb.tile([128, 1024], F32, tag="big",
                                  name=f"s_{qc}_{hp}_{kc}")
                    for hh in range(2):
                        rows = slice(64 * hh, 64 * (hh + 1))
                        nc.tensor.matmul(
                            ps[:, 512 * hh + off:512 * (hh + 1)],
                            kT[rows, hp, 128 * kc:128 * (kc + 1)],
                            qT[rows, hp, 512 * qc + off:512 * (qc + 1)],
                            start=True, stop=True,
                            tile_position=(64 * hh, 0))
                    if 0 <= t <= 3:
                        mvs = []
                        for hh in range(2):
                            mh = mhp.tile([128, 512], BF16, tag="mh")
                            nc.scalar.activation(
                                mh[:, off:512],
                                ps[:, 512 * hh + off:512 * (hh + 1)], AF.Exp)
                            tri = slice(off, off + 128)
                            nc.vector.tensor_tensor(
                                mh[:, tri], mh[:, tri], msk[:, t, tri], MULT)
                            mvs.append((mh, 0))
                        pts[kc] = mvs
                    else:
                        pt = ptp.tile([128, 1024], BF16, tag="pt")
                        nc.scalar.activation(pt[:], ps[:], AF.Exp)
                        pts[kc] = [(pt, 0), (pt, 512)]

                def av(kc):
                    off = offs[kc]
                    for hh in range(2):
                        if hh not in po_t:
                            po_t[hh] = pop.tile([65, 512], F32, tag="po",
                                                name=f"po_{qc}_{hp}_{hh}")
                        h = 2 * hp + hh
                        src, c0 = pts[kc][hh]
                        nc.tensor.matmul(
                            po_t[hh][:, off:512],
                            vp[:, kc, 65 * h:65 * (h + 1)],
                            src[:, c0 + off:c0 + 512],
                            start=(kc == 0), stop=(kc == K - 1))

                LAG = 2
                for kc in range(K):
                    if kc >= LAG:
                        av(kc - LAG)
                    qkt(kc)
                    if kc == 2:
                        flush_den()
                    if kc in slots:
                        pop_fill(slots[kc])
                flush_fins()  # vp writes must precede the trailing A@V reads
                av(K - 2)
                av(K - 1)

                for hh in range(2):
                    pu = pup.tile([65, 512], F32, tag="pu")
                    nc.vector.tensor_copy(out=pu[:], in_=po_t[hh][:])
                    nc.sync.dma_start(scr_d[hp, qc, hh, :], pu[64:65, :])
                    rb = rbp.tile([64, 512], F32, tag="rb")
                    sl = scr_d[hp, qc, hh, :]
                    bc = bass.AP(tensor=sl.tensor, offset=sl.offset,
                                 ap=[[0, 64]] + list(sl.ap))
                    nc.sync.dma_start(rb[:], bc)
                    pend_den.append((hp, qc, hh, pu, rb))

            # ---- phase A: warm-up ----
            for st, half in ((0, 0), (0, 1), (1, 0), (1, 1)):
                VP(st, half, "v")()
            PJ(wk_sb, kT, 0, 0, "v")()
            PJ(wk_sb, kT, 0, 1, "v")()
            PJ(wq_sb, qT, 0, 0, "v")()

            fillers += [mk_vp(2, 0, "s"), mk_vp(2, 1, "s"),
                        mk_vp(3, 0, "s"), mk_vp(3, 1, "s")]
            for nt in range(1, NT):
                fillers += [mk_pj("q", nt, 0, "s"), mk_pj("k", nt, 0, "s"),
                            mk_pj("k", nt, 1, "s")]
            fillers += [mk_vp(st, h, "s") for st in (4, 5, 6, 7)
                        for h in (0, 1)]
            fillers += [mk_pj("q", nt, 1, "s") for nt in range(NT)]

            # ---- qc0 pass ----
            for hp in range(NT):
                if hp > 0:
                    force(f"q{hp}s0", f"k{hp}s0", f"k{hp}s1")
                emit_att(0, hp, slots={0: 1, 1: 2, 2: 1, 3: 1})

            # SE units become available once qc0 outT is complete
            fillers += [mk_se(st, h, "v") for st in (0, 1, 2, 3)
                        for h in (0, 1)]

            # ---- qc1 pass ----
            for hp in range(NT):
                force(f"q{hp}s1")
                emit_att(1, hp, slots={1: 1, 3: 1, 5: 1})

            # ---- tail ----
            while fillers:
                pop_fill(1)
            flush_fins()
            flush_den()
            eng = ["v", "s"]
            for i, (st, half) in enumerate(
                    [(st, h) for st in (4, 5, 6, 7) for h in (0, 1)]):
                fin = SE(st, half, eng[i % 2])
                flush_fins()
                pend_fin.append(fin)
            flush_fins()
            if DEBUG_DUMP:
                nc.sync.dma_start(dbg_q[:], qT[:])
                nc.sync.dma_start(dbg_k[:], kT[:])
                nc.sync.dma_start(dbg_v[:], vp[:])
                nc.sync.dma_start(dbg_o[:], outT[:])

    nc.compile()
    return nc


def _get_compiled():
    global _compiled
    if _compiled is None:
        _compiled = _build_nc()
    return _compiled


def _prep_inputs(x, W_attn, W_proj):
    import ml_dtypes

    bf16 = ml_dtypes.bfloat16
    x = np.asarray(x, dtype=np.float32)
    W_attn = np.asarray(W_attn, dtype=np.float32)
    W_proj = np.asarray(W_proj, dtype=np.float32)

    xT = np.ascontiguousarray(np.transpose(x, (0, 2, 1))).astype(bf16)
    wq = (np.ascontiguousarray(W_attn[:, 0:D]) * np.float32(0.125)).astype(bf16)
    wk = np.ascontiguousarray(W_attn[:, D:2 * D]).astype(bf16)
    wv = np.ascontiguousarray(W_attn[:, 2 * D:3 * D]).astype(bf16)
    wp = W_proj.astype(bf16)
    masks = _build_masks()
    return [
        {"xT": xT[b], "wq": wq, "wk": wk, "wv": wv, "wp": wp, "masks": masks}
        for b in range(B)
    ]


def kernel(x, W_attn, W_proj):
    from concourse.bass_utils import run_bass_kernel_spmd

    nc = _get_compiled()
    in_maps = _prep_inputs(x, W_attn, W_proj)
    res = run_bass_kernel_spmd(nc, in_maps, list(range(NC_)))
    y = np.stack([res.results[b]["y"] for b in range(B)], axis=0)
    return y.astype(np.float32)
